# revision 29
# baseline (speedup 1.0000x reference)
"""GQA attention (B=2, S=2048, D=2048, H=16, KV=4, HD=128) with RoPE + causal
softmax + output projection, on 8 TRN2 NeuronCores.

Sharding: B x KV = 2 x 4 = 8 perfectly balanced shards. Core c handles batch
c//4 and kv-group c%4 (4 q heads + 1 kv head). wq/wk/wv are column-sharded,
wo row-sharded; the 4 partial wo outputs per batch are summed on the host
(the unshard step for a row-sharded matmul).

Design (v3, from the v1 306us baseline; ~252us clock-normalized traced):
  - x is transposed on the HOST, so x^T tiles arrive via plain contiguous
    DMA (2KB packets) instead of the transpose crossbar (256B packets).
    v1 lost ~30us of prologue and a ~30us mid-kernel stall to transpose
    DMA contention.
  - prologue DMAs are ordered by stage2(0)'s h-major consumption (wq h0,
    x^T block 0, cs, wq h1..h3, wkv); DMA queue completion is in-order
    and Sync posts ~0.7us/DMA, so order is everything. First matmul ~11us
    (7us of that is fixed framework init).
  - V tiles ([s,hd] layout) produced by PE identity-transposes of the
    bf16 pv projection (start/stop groups into one PSUM bank), then one
    ACT copy per block to SBUF. No SBUF->SBUF transpose DMA.
  - causal mask via a PE-accumulated mneg^T (-1e5 above diagonal) matmul
    onto the diagonal score strip, keeping DVE/GpSimd off the
    scores->exp->l/PV critical path; exp then underflows masked to 0.
  - full-bf16 matmul pipeline (f32 PSUM accumulation), exp on ACT with no
    max subtraction (scores are O(1) by construction).
  - l via all-ones [128,128] matmul (broadcasts sum_k P into all
    partitions, PSUM-accumulated). Off-diagonal P tiles are quad-summed
    on the DVE first, so the ones-matmul streams 1/4 of the columns
    (PE cols for l: 69.6k -> 32.8k per core).
  - causal width trim: diagonal score tiles only compute q >= 128*r.
  - score matmuls prefetch DEPTH=4 items ahead ACROSS head boundaries
    (no per-head pipeline refill); ACT exp is the stage3 near-bottleneck
    so stage4's PSUM->SBUF casts all go to DVE.
  - out partials stored/DMA'd as bf16 (halves output traffic); host
    accumulates the 4 row-shard partials in f32.
  - PSUM banks: proj(2, shared w/ V-transpose + stage4) + o(1) + st(4) +
    l(1) = 8.
  - NOTE: run-to-run HW exec varies ~±10% with the chip's DVFS state
    (matmul slice 379ns vs 454ns mode); compare min-of-N or
    clock-normalized.
"""
import os
import sys

import numpy as np

if "/opt/trn_rl_repo" not in sys.path:
    sys.path.insert(0, "/opt/trn_rl_repo")

B, S, D = 2, 2048, 2048
H, KV, HD = 16, 4, 128
NREP = H // KV            # 4 q heads per core
EG = NREP * HD            # 512: per-core q width
NC_CORES = 8
SB = 4                    # seq blocks of 512
ST = 4                    # 128-row seq tiles per block
DT = D // 128             # 16 contraction tiles
SCALE = float(1.0 / np.sqrt(HD))

_CACHE = {}
LAST_RESULT = None        # BassKernelResults of the most recent run (for test.py)


def _install_trace_shim():
    """antenv.axon_hooks is missing in this image; run_bass_kernel_spmd's
    trace path needs it. Also neuter the S3 artifact upload."""
    import types

    try:
        import antenv.axon_hooks  # noqa: F401
    except ImportError:
        try:
            import antenv
            from trn_agent_boot.trn_boot import _ntff_profile_via_ctypes

            mod = types.ModuleType("antenv.axon_hooks")
            _hook = [None]
            mod.set_axon_ntff_profile_hook = lambda h: _hook.__setitem__(0, h)
            mod.get_axon_ntff_profile_hook = lambda: _hook[0]
            sys.modules["antenv.axon_hooks"] = mod
            antenv.axon_hooks = mod
            mod.set_axon_ntff_profile_hook(
                _ntff_profile_via_ctypes("/opt/axon/libaxon_pjrt.so")
            )
        except Exception:
            return
    import concourse.bass_utils as bu

    bu.upload_artifacts = lambda tmpdir: f"local:{tmpdir}"


def _build():
    import concourse.mybir as mybir
    import concourse.tile as tile
    from concourse import bacc

    f32 = mybir.dt.float32
    bf16 = mybir.dt.bfloat16
    EXP = mybir.ActivationFunctionType.Exp

    nc = bacc.Bacc(None, target_bir_lowering=False)
    xT_d = nc.declare_dram_parameter("xT", [D, S], bf16, isOutput=False)
    wq_d = nc.declare_dram_parameter("wq", [D, EG], bf16, isOutput=False)
    wkv_d = nc.declare_dram_parameter("wkv", [D, 2 * HD], bf16, isOutput=False)
    wo_d = nc.declare_dram_parameter("wo", [EG, D], bf16, isOutput=False)
    cs_d = nc.declare_dram_parameter("csT", [128, S], bf16, isOutput=False)
    moi_d = nc.declare_dram_parameter("moi", [128, 384], bf16, isOutput=False)
    out_d = nc.declare_dram_parameter("out", [S, D], bf16, isOutput=True)

    with tile.TileContext(nc) as tc:
        with (
            tc.tile_pool(name="fixed", bufs=1) as fixed,
            tc.tile_pool(name="xt", bufs=4) as xtp,
            tc.tile_pool(name="qt", bufs=2) as qtp,
            tc.tile_pool(name="ot", bufs=2) as otp,
            tc.tile_pool(name="pt", bufs=6) as ptp,
            tc.tile_pool(name="rope", bufs=3) as ropep,
            tc.tile_pool(name="pp", bufs=3) as ppp,
            tc.tile_pool(name="vt", bufs=2) as vtp,
            tc.tile_pool(name="r", bufs=2) as rp,
            tc.tile_pool(name="ob", bufs=3) as obp,
            # PSUM banks: proj(2) + o(1) + st(4) + l(1) = 8
            tc.tile_pool(name="psA", bufs=2, space="PSUM") as psA,
            tc.tile_pool(name="psS", bufs=4, space="PSUM") as psS,
            tc.tile_pool(name="psB", bufs=1, space="PSUM") as psB,
        ):
            # ---- persistent tiles (DMAs emitted in the ordered prologue) ----
            wq_s = fixed.tile([128, DT, EG], bf16)
            wkv_s = fixed.tile([128, DT, 2 * HD], bf16)
            wo_s = fixed.tile([128, NREP, D], bf16)
            # moi = mneg^T (strict upper-tri -1e5) | all-ones | identity.
            # mneg^T is PE-accumulated onto diagonal score strips (mask add on
            # the PE keeps the mask off the DVE/ACT critical path). The
            # l-matmul with ones broadcasts sum_k P into every output
            # partition at the same cost as an M=1 matmul (cost ~ N), making
            # 1/l directly consumable by the O^T normalize multiply.
            moi_s = fixed.tile([128, 384], bf16)
            mnegT_s = moi_s[:, 0:128]
            ones_s = moi_s[:, 128:256]
            ident_s = moi_s[:, 256:384]
            cs_s = fixed.tile([128, S], bf16)       # cos^T | sin^T, all blocks
            KT = fixed.tile([128, SB, 512], bf16)   # rotated K^T [hd, s]
            V = fixed.tile([128, DT, HD], bf16)     # V [s%128, s-tile, hd]

            def rope(dst, psrc, cs):
                """dst[128,512] bf16 = rotate(psrc[128,512] PSUM f32).
                Rows 0:64 = real half, 64:128 = imag half (pre-permuted
                weights); cs rows 0:64 = cos^T, 64:128 = sin^T. Multiplies
                on DVE (PSUM reads), add/sub on GpSimd (SBUF only)."""
                re, im = psrc[0:64, :], psrc[64:128, :]
                co, si = cs[0:64, :], cs[64:128, :]
                t1 = ropep.tile([64, 512], bf16, tag="t1")
                nc.vector.tensor_mul(t1, re, co)
                t2 = ropep.tile([64, 512], bf16, tag="t2")
                nc.vector.tensor_mul(t2, im, si)
                nc.gpsimd.tensor_sub(dst[0:64, :], t1, t2)
                t3 = ropep.tile([64, 512], bf16, tag="t1")
                nc.vector.tensor_mul(t3, re, si)
                t4 = ropep.tile([64, 512], bf16, tag="t2")
                nc.vector.tensor_mul(t4, im, co)
                nc.gpsimd.tensor_add(dst[64:128, :], t3, t4)

            xT_ap = xT_d.ap().rearrange("(t k) s -> k t s", k=128)

            def load_xt(sb, nchunk):
                """x^T for block sb: plain contiguous DMA (host transposed)."""
                xt = xtp.tile([128, DT, 512], bf16, tag="xt")
                step = DT // nchunk
                for dg in range(nchunk):
                    nc.sync.dma_start(
                        xt[:, dg * step : (dg + 1) * step, :],
                        xT_ap[
                            :, dg * step : (dg + 1) * step, sb * 512 : (sb + 1) * 512
                        ],
                    )
                return xt

            def stage2(sb, xt):
                """Q^T/K^T/V projections + RoPE for block sb."""
                cs = cs_s[:, sb * 512 : (sb + 1) * 512]
                qt = qtp.tile([128, NREP, 512], bf16, tag="qt")
                for h in range(NREP):
                    pq = psA.tile([128, 512], f32, tag="proj")
                    for dt in range(DT):
                        nc.tensor.matmul(
                            pq,
                            wq_s[:, dt, h * 128 : (h + 1) * 128],
                            xt[:, dt, :],
                            start=(dt == 0),
                            stop=(dt == DT - 1),
                        )
                    rope(qt[:, h, :], pq, cs)

                pk = psA.tile([128, 512], f32, tag="proj")
                for dt in range(DT):
                    nc.tensor.matmul(
                        pk, wkv_s[:, dt, 0:HD], xt[:, dt, :],
                        start=(dt == 0), stop=(dt == DT - 1),
                    )
                rope(KT[:, sb, :], pk, cs)

                pv = psA.tile([128, 512], f32, tag="proj")
                for dt in range(DT):
                    nc.tensor.matmul(
                        pv, wkv_s[:, dt, HD : 2 * HD], xt[:, dt, :],
                        start=(dt == 0), stop=(dt == DT - 1),
                    )
                vt_tmp = vtp.tile([128, 512], bf16, tag="vt")
                nc.vector.tensor_copy(vt_tmp, pv)
                # PE identity-transpose pv^T -> V[s,hd] tiles (one PSUM bank,
                # 4 independent start/stop groups into disjoint regions).
                vps = psA.tile([128, ST, HD], bf16, tag="proj")
                for c in range(ST):
                    nc.tensor.transpose(
                        vps[:, c, :], vt_tmp[:, c * 128 : (c + 1) * 128], ident_s
                    )
                nc.scalar.copy(V[:, sb * ST : (sb + 1) * ST, :], vps)
                return qt

            def stage3(sb, qt):
                """Causal attention for q-block sb, all 4 heads.
                Emission is software-pipelined: DEPTH score matmuls run ahead
                of the exp->l/PV chain so the in-order PE stream never stalls
                on ACT latency. Off-diagonal P tiles are pair-summed on the
                DVE so the l (ones) matmul streams half the columns."""
                ot = otp.tile([128, NREP, 512], bf16, tag="ot")
                nkt = (sb + 1) * ST
                DEPTH = 4

                def kt_geo(kt):
                    """Valid q range for k-tile kt in this q-block: diagonal
                    tiles only cover q >= 128*r (causal width trim)."""
                    r = kt - sb * ST
                    qo = 128 * r if r > 0 else 0
                    return r, qo

                def emit_st(h, kt):
                    r, qo = kt_geo(kt)
                    pst = psS.tile([128, 512], f32, tag="st")
                    nc.tensor.matmul(
                        pst[:, qo:],
                        KT[:, kt // ST, (kt % ST) * 128 : (kt % ST + 1) * 128],
                        qt[:, h, qo:],
                        start=True, stop=(r < 0),
                    )
                    if r >= 0:
                        # causal mask: PE-accumulate mneg^T (-1e5 above the
                        # diagonal) onto the 128-col strip; exp then
                        # underflows the masked entries to 0
                        nc.tensor.matmul(
                            pst[:, qo : qo + 128], mnegT_s, ident_s,
                            start=False, stop=True, skip_group_check=True,
                        )
                    return pst

                # score prefetch runs DEPTH items ahead ACROSS head
                # boundaries, so head transitions have no pipeline refill
                items = [(h, kt) for h in range(NREP) for kt in range(nkt)]
                sts = {}
                ahead = 0

                def prefetch(upto):
                    nonlocal ahead
                    while ahead < len(items) and ahead < upto:
                        sts[items[ahead]] = emit_st(*items[ahead])
                        ahead += 1

                po = pl = None
                lfirst = True
                quad = []
                for idx, (h, kt) in enumerate(items):
                    if kt == 0:
                        po = psA.tile([128, 512], f32, tag="o", bufs=1)
                        pl = psB.tile([128, 512], f32, tag="l")
                        lfirst = True
                        quad = []
                    prefetch(idx + DEPTH)
                    r, qo = kt_geo(kt)
                    pst = sts.pop((h, kt))
                    pt = ptp.tile([128, 512], bf16, tag="pt")
                    nc.scalar.activation(pt[:, qo:], pst[:, qo:], EXP, scale=SCALE)
                    if r < 0:
                        # off-diagonal: quad-sum P tiles on the DVE so the
                        # ones (l) matmul streams 1/4 of the columns
                        quad.append(pt)
                        if len(quad) == 4:
                            p01 = ppp.tile([128, 512], bf16, tag="pp")
                            nc.vector.tensor_add(p01, quad[0], quad[1])
                            p23 = ppp.tile([128, 512], bf16, tag="pp")
                            nc.vector.tensor_add(p23, quad[2], quad[3])
                            p03 = ppp.tile([128, 512], bf16, tag="pp")
                            nc.vector.tensor_add(p03, p01, p23)
                            quad = []
                            nc.tensor.matmul(
                                pl, ones_s, p03, start=lfirst, stop=False,
                            )
                            lfirst = False
                    else:
                        nc.tensor.matmul(
                            pl[:, qo:], ones_s, pt[:, qo:],
                            start=lfirst, stop=(kt == nkt - 1),
                        )
                        lfirst = False
                    nc.tensor.matmul(
                        po[:, qo:], V[:, kt, :], pt[:, qo:],
                        start=(kt == 0), stop=(kt == nkt - 1),
                    )
                    if kt == nkt - 1:
                        rb = rp.tile([128, 512], f32, tag="rb")
                        nc.vector.reciprocal_approx_fast(out=rb, in_=pl)
                        nc.vector.tensor_mul(ot[:, h, :], po, rb)
                return ot

            def stage4(sb, ot):
                """Output projection for q-block sb (bf16 partials out).
                PSUM->SBUF casts alternate DVE/ACT; one out DMA per row."""
                for st in range(ST):
                    ob = obp.tile([128, 4, 512], bf16, tag="ob")
                    for db in range(4):
                        pw = psA.tile([128, 512], f32, tag="proj")
                        for h in range(NREP):
                            nc.tensor.matmul(
                                pw,
                                ot[:, h, st * 128 : (st + 1) * 128],
                                wo_s[:, h, db * 512 : (db + 1) * 512],
                                start=(h == 0), stop=(h == NREP - 1),
                            )
                        nc.vector.tensor_copy(ob[:, db, :], pw)
                    row0 = (sb * ST + st) * 128
                    nc.sync.dma_start(
                        out_d.ap()[row0 : row0 + 128, 0:1024], ob[:, 0:2, :]
                    )
                    nc.sync.dma_start(
                        out_d.ap()[row0 : row0 + 128, 1024:2048], ob[:, 2:4, :]
                    )

            # ---- ordered DMA prologue. DMA queue completion is in-order and
            # Sync posting is ~0.7us/DMA, so deliveries are ordered to match
            # stage2(0)'s consumption: wq h0 + x^T block 0 first, then cs,
            # then wq h1..h3 (one per ~6us of h-loop), then wkv. ----
            wq_ap = wq_d.ap().rearrange("(t k) e -> k t e", k=128)

            def load_wq(h):
                nc.sync.dma_start(
                    wq_s[:, :, h * 128 : (h + 1) * 128],
                    wq_ap[:, :, h * 128 : (h + 1) * 128],
                )

            # delivery ordered by stage2(0)'s h-major consumption, with the
            # h0 pass's inputs (wq h0, x^T tiles) split into 256KB chunks so
            # the first matmul starts ~9.5us and the h0 pass tracks delivery;
            # then wq h1..h3 (one per ~6us of h-loop), wkv for the K/V passes
            nc.sync.dma_start(wq_s[:, 0:8, 0:128], wq_ap[:, 0:8, 0:128])
            xt0 = xtp.tile([128, DT, 512], bf16, tag="xt")
            for dg in range(3):
                nc.sync.dma_start(
                    xt0[:, 2 * dg : 2 * dg + 2, :],
                    xT_ap[:, 2 * dg : 2 * dg + 2, 0:512],
                )
            nc.sync.dma_start(wq_s[:, 8:16, 0:128], wq_ap[:, 8:16, 0:128])
            for dg in range(3, 8):
                nc.sync.dma_start(
                    xt0[:, 2 * dg : 2 * dg + 2, :],
                    xT_ap[:, 2 * dg : 2 * dg + 2, 0:512],
                )
            xts = [xt0]
            load_wq(1)
            nc.sync.dma_start(cs_s, cs_d.ap())
            load_wq(2)
            load_wq(3)
            nc.sync.dma_start(wkv_s, wkv_d.ap().rearrange("(t k) e -> k t e", k=128))
            nc.sync.dma_start(moi_s, moi_d.ap())
            xts.append(load_xt(1, 2))
            xts.append(load_xt(2, 2))
            xts.append(load_xt(3, 2))
            wo_ap = wo_d.ap().rearrange("(h k) n -> k h n", k=128)
            for i in range(2):
                nc.sync.dma_start(
                    wo_s[:, :, 1024 * i : 1024 * i + 1024],
                    wo_ap[:, :, 1024 * i : 1024 * i + 1024],
                )
            # Software-pipelined outer loop: projections for block sb+1 are
            # emitted BEFORE the wo-stage of block sb, so the in-order PE
            # stream has independent matmuls to run while block sb's
            # normalization tail completes.
            qt = stage2(0, xts[0])
            for sb in range(SB):
                ot = stage3(sb, qt)
                if sb + 1 < SB:
                    qt = stage2(sb + 1, xts[sb + 1])
                stage4(sb, ot)
    nc.finalize()
    return nc


def _get_nc():
    if "nc" not in _CACHE:
        _CACHE["nc"] = _build()
    return _CACHE["nc"]


def _host_prep(x, wq, wk, wv, wo, freqs_cos, freqs_sin):
    """Build the 8 per-core input maps (bf16 casts + x transpose on host)."""
    import ml_dtypes

    bf = ml_dtypes.bfloat16
    perm = np.concatenate([np.arange(0, HD, 2), np.arange(1, HD, 2)])  # even|odd
    csT = np.concatenate(
        [np.ascontiguousarray(freqs_cos.T), np.ascontiguousarray(freqs_sin.T)], axis=0
    ).astype(bf)  # [128, S]
    # mneg^T[i,j] = -1e5 strictly above the diagonal (masked, q < k within
    # the diagonal strip after the PE transpose-accumulate), else 0
    ii = np.arange(128, dtype=np.int64)[:, None]
    jj = np.arange(128, dtype=np.int64)[None, :]
    mnegT = np.where(ii >= jj, 0.0, -1e5).astype(np.float32)
    moi = np.concatenate(
        [mnegT, np.ones((128, 128), np.float32), np.eye(128, dtype=np.float32)],
        axis=1,
    ).astype(bf)

    xT = [np.ascontiguousarray(x[b].astype(bf).T) for b in range(B)]
    in_maps = []
    for c in range(NC_CORES):
        b, g = divmod(c, NREP)
        wq_g = wq[:, g * EG : (g + 1) * EG].copy()
        for h in range(NREP):
            blk = wq_g[:, h * HD : (h + 1) * HD]
            wq_g[:, h * HD : (h + 1) * HD] = blk[:, perm]
        wk_g = wk[:, g * HD : (g + 1) * HD][:, perm]
        wv_g = wv[:, g * HD : (g + 1) * HD]
        wkv_g = np.concatenate([wk_g, wv_g], axis=1)
        wo_g = wo[g * EG : (g + 1) * EG, :]
        in_maps.append(
            {
                "xT": xT[b],
                "wq": np.ascontiguousarray(wq_g).astype(bf),
                "wkv": np.ascontiguousarray(wkv_g).astype(bf),
                "wo": np.ascontiguousarray(wo_g).astype(bf),
                "csT": csT,
                "moi": moi,
            }
        )
    return in_maps


def kernel(x, wq, wk, wv, wo, freqs_cos, freqs_sin):
    global LAST_RESULT
    from concourse.bass_utils import run_bass_kernel_spmd

    trace = bool(int(os.environ.get("BASS_KERNEL_TRACE", "0")))
    if trace:
        _install_trace_shim()

    x = np.asarray(x, dtype=np.float32)
    wq = np.asarray(wq, dtype=np.float32)
    wk = np.asarray(wk, dtype=np.float32)
    wv = np.asarray(wv, dtype=np.float32)
    wo = np.asarray(wo, dtype=np.float32)
    freqs_cos = np.asarray(freqs_cos, dtype=np.float32)
    freqs_sin = np.asarray(freqs_sin, dtype=np.float32)

    nc = _get_nc()
    in_maps = _host_prep(x, wq, wk, wv, wo, freqs_cos, freqs_sin)
    res = run_bass_kernel_spmd(nc, in_maps, list(range(NC_CORES)), trace=trace)
    LAST_RESULT = res

    out = np.empty((B, S, D), dtype=np.float32)
    for b in range(B):
        acc = res.results[b * NREP]["out"].astype(np.float32, copy=True)
        for g in range(1, NREP):
            acc += res.results[b * NREP + g]["out"].astype(np.float32)
        out[b] = acc
    return out


# revision 33
# speedup vs baseline: 1.0215x; 1.0215x over previous
"""GQA attention (B=2, S=2048, D=2048, H=16, KV=4, HD=128) with RoPE + causal
softmax + output projection, on 8 TRN2 NeuronCores.

Sharding: B x KV = 2 x 4 = 8 perfectly balanced shards. Core c handles batch
c//4 and kv-group c%4 (4 q heads + 1 kv head). wq/wk/wv are column-sharded,
wo row-sharded; the 4 partial wo outputs per batch are summed on the host
(the unshard step for a row-sharded matmul).

Design (v3, from the v1 306us baseline; ~252us clock-normalized traced):
  - x is transposed on the HOST, so x^T tiles arrive via plain contiguous
    DMA (2KB packets) instead of the transpose crossbar (256B packets).
    v1 lost ~30us of prologue and a ~30us mid-kernel stall to transpose
    DMA contention.
  - prologue DMAs are ordered by stage2(0)'s h-major consumption (wq h0,
    x^T block 0, cs, wq h1..h3, wkv); DMA queue completion is in-order
    and Sync posts ~0.7us/DMA, so order is everything. First matmul ~11us
    (7us of that is fixed framework init).
  - V tiles ([s,hd] layout) produced by PE identity-transposes of the
    bf16 pv projection (start/stop groups into one PSUM bank), then one
    ACT copy per block to SBUF. No SBUF->SBUF transpose DMA.
  - causal mask via a PE-accumulated mneg^T (-1e5 above diagonal) matmul
    onto the diagonal score strip, keeping DVE/GpSimd off the
    scores->exp->l/PV critical path; exp then underflows masked to 0.
  - full-bf16 matmul pipeline (f32 PSUM accumulation), exp on ACT with no
    max subtraction (scores are O(1) by construction).
  - l via all-ones [128,128] matmul (broadcasts sum_k P into all
    partitions, PSUM-accumulated). Off-diagonal P tiles are quad-summed
    on the DVE first, so the ones-matmul streams 1/4 of the columns
    (PE cols for l: 69.6k -> 32.8k per core).
  - causal width trim: diagonal score tiles only compute q >= 128*r.
  - score matmuls prefetch DEPTH=4 items ahead ACROSS head boundaries
    (no per-head pipeline refill); ACT exp is the stage3 near-bottleneck
    so stage4's PSUM->SBUF casts all go to DVE.
  - out partials stored/DMA'd as bf16 (halves output traffic); host
    accumulates the 4 row-shard partials in f32.
  - PSUM banks: proj(2, shared w/ V-transpose + stage4) + o(1) + st(4) +
    l(1) = 8.
  - NOTE: run-to-run HW exec varies ~±10% with the chip's DVFS state
    (matmul slice 379ns vs 454ns mode); compare min-of-N or
    clock-normalized.
"""
import os
import sys

import numpy as np

if "/opt/trn_rl_repo" not in sys.path:
    sys.path.insert(0, "/opt/trn_rl_repo")

B, S, D = 2, 2048, 2048
H, KV, HD = 16, 4, 128
NREP = H // KV            # 4 q heads per core
EG = NREP * HD            # 512: per-core q width
NC_CORES = 8
SB = 4                    # seq blocks of 512
ST = 4                    # 128-row seq tiles per block
DT = D // 128             # 16 contraction tiles
SCALE = float(1.0 / np.sqrt(HD))

_CACHE = {}
LAST_RESULT = None        # BassKernelResults of the most recent run (for test.py)


def _install_trace_shim():
    """antenv.axon_hooks is missing in this image; run_bass_kernel_spmd's
    trace path needs it. Also neuter the S3 artifact upload."""
    import types

    try:
        import antenv.axon_hooks  # noqa: F401
    except ImportError:
        try:
            import antenv
            from trn_agent_boot.trn_boot import _ntff_profile_via_ctypes

            mod = types.ModuleType("antenv.axon_hooks")
            _hook = [None]
            mod.set_axon_ntff_profile_hook = lambda h: _hook.__setitem__(0, h)
            mod.get_axon_ntff_profile_hook = lambda: _hook[0]
            sys.modules["antenv.axon_hooks"] = mod
            antenv.axon_hooks = mod
            mod.set_axon_ntff_profile_hook(
                _ntff_profile_via_ctypes("/opt/axon/libaxon_pjrt.so")
            )
        except Exception:
            return
    import concourse.bass_utils as bu

    bu.upload_artifacts = lambda tmpdir: f"local:{tmpdir}"


def _build():
    import concourse.mybir as mybir
    import concourse.tile as tile
    from concourse import bacc

    f32 = mybir.dt.float32
    bf16 = mybir.dt.bfloat16
    EXP = mybir.ActivationFunctionType.Exp

    nc = bacc.Bacc(None, target_bir_lowering=False)
    xT_d = nc.declare_dram_parameter("xT", [D, S], bf16, isOutput=False)
    wq_d = nc.declare_dram_parameter("wq", [D, EG], bf16, isOutput=False)
    wkv_d = nc.declare_dram_parameter("wkv", [D, 2 * HD], bf16, isOutput=False)
    wo_d = nc.declare_dram_parameter("wo", [EG, D], bf16, isOutput=False)
    cs_d = nc.declare_dram_parameter("csT", [128, S], bf16, isOutput=False)
    moi_d = nc.declare_dram_parameter("moi", [128, 384], bf16, isOutput=False)
    out_d = nc.declare_dram_parameter("out", [S, D], bf16, isOutput=True)

    with tile.TileContext(nc) as tc:
        with (
            tc.tile_pool(name="fixed", bufs=1) as fixed,
            tc.tile_pool(name="xt", bufs=4) as xtp,
            tc.tile_pool(name="qt", bufs=2) as qtp,
            tc.tile_pool(name="ot", bufs=2) as otp,
            tc.tile_pool(name="pt", bufs=6) as ptp,
            tc.tile_pool(name="rope", bufs=3) as ropep,
            tc.tile_pool(name="pp", bufs=3) as ppp,
            tc.tile_pool(name="vt", bufs=2) as vtp,
            tc.tile_pool(name="r", bufs=2) as rp,
            tc.tile_pool(name="ob", bufs=3) as obp,
            # PSUM banks: proj(2) + o(1) + st(4) + l(1) = 8
            tc.tile_pool(name="psA", bufs=2, space="PSUM") as psA,
            tc.tile_pool(name="psS", bufs=4, space="PSUM") as psS,
            tc.tile_pool(name="psB", bufs=1, space="PSUM") as psB,
        ):
            # ---- persistent tiles (DMAs emitted in the ordered prologue) ----
            wq_s = fixed.tile([128, DT, EG], bf16)
            wkv_s = fixed.tile([128, DT, 2 * HD], bf16)
            wo_s = fixed.tile([128, NREP, D], bf16)
            # moi = mneg^T (strict upper-tri -1e5) | all-ones | identity.
            # mneg^T is PE-accumulated onto diagonal score strips (mask add on
            # the PE keeps the mask off the DVE/ACT critical path). The
            # l-matmul with ones broadcasts sum_k P into every output
            # partition at the same cost as an M=1 matmul (cost ~ N), making
            # 1/l directly consumable by the O^T normalize multiply.
            moi_s = fixed.tile([128, 384], bf16)
            mnegT_s = moi_s[:, 0:128]
            ones_s = moi_s[:, 128:256]
            ident_s = moi_s[:, 256:384]
            cs_s = fixed.tile([128, S], bf16)       # cos^T | sin^T, all blocks
            KT = fixed.tile([128, SB, 512], bf16)   # rotated K^T [hd, s]
            V = fixed.tile([128, DT, HD], bf16)     # V [s%128, s-tile, hd]

            def rope(dst, psrc, cs):
                """dst[128,512] bf16 = rotate(psrc[128,512] PSUM f32).
                Rows 0:64 = real half, 64:128 = imag half (pre-permuted
                weights); cs rows 0:64 = cos^T, 64:128 = sin^T. Multiplies
                on DVE (PSUM reads), add/sub on GpSimd (SBUF only)."""
                re, im = psrc[0:64, :], psrc[64:128, :]
                co, si = cs[0:64, :], cs[64:128, :]
                t1 = ropep.tile([64, 512], bf16, tag="t1")
                nc.vector.tensor_mul(t1, re, co)
                t2 = ropep.tile([64, 512], bf16, tag="t2")
                nc.vector.tensor_mul(t2, im, si)
                nc.gpsimd.tensor_sub(dst[0:64, :], t1, t2)
                t3 = ropep.tile([64, 512], bf16, tag="t1")
                nc.vector.tensor_mul(t3, re, si)
                t4 = ropep.tile([64, 512], bf16, tag="t2")
                nc.vector.tensor_mul(t4, im, co)
                nc.gpsimd.tensor_add(dst[64:128, :], t3, t4)

            xT_ap = xT_d.ap().rearrange("(t k) s -> k t s", k=128)

            def load_xt(sb, nchunk):
                """x^T for block sb: plain contiguous DMA (host transposed)."""
                xt = xtp.tile([128, DT, 512], bf16, tag="xt")
                step = DT // nchunk
                for dg in range(nchunk):
                    nc.sync.dma_start(
                        xt[:, dg * step : (dg + 1) * step, :],
                        xT_ap[
                            :, dg * step : (dg + 1) * step, sb * 512 : (sb + 1) * 512
                        ],
                    )
                return xt

            def stage2(sb, xt):
                """Q^T/K^T/V projections + RoPE for block sb."""
                cs = cs_s[:, sb * 512 : (sb + 1) * 512]
                qt = qtp.tile([128, NREP, 512], bf16, tag="qt")
                for h in range(NREP):
                    pq = psA.tile([128, 512], f32, tag="proj")
                    for dt in range(DT):
                        nc.tensor.matmul(
                            pq,
                            wq_s[:, dt, h * 128 : (h + 1) * 128],
                            xt[:, dt, :],
                            start=(dt == 0),
                            stop=(dt == DT - 1),
                        )
                    rope(qt[:, h, :], pq, cs)

                pk = psA.tile([128, 512], f32, tag="proj")
                for dt in range(DT):
                    nc.tensor.matmul(
                        pk, wkv_s[:, dt, 0:HD], xt[:, dt, :],
                        start=(dt == 0), stop=(dt == DT - 1),
                    )
                rope(KT[:, sb, :], pk, cs)

                pv = psA.tile([128, 512], f32, tag="proj")
                for dt in range(DT):
                    nc.tensor.matmul(
                        pv, wkv_s[:, dt, HD : 2 * HD], xt[:, dt, :],
                        start=(dt == 0), stop=(dt == DT - 1),
                    )
                vt_tmp = vtp.tile([128, 512], bf16, tag="vt")
                nc.vector.tensor_copy(vt_tmp, pv)
                # PE identity-transpose pv^T -> V[s,hd] tiles (one PSUM bank,
                # 4 independent start/stop groups into disjoint regions).
                vps = psA.tile([128, ST, HD], bf16, tag="proj")
                for c in range(ST):
                    nc.tensor.transpose(
                        vps[:, c, :], vt_tmp[:, c * 128 : (c + 1) * 128], ident_s
                    )
                nc.scalar.copy(V[:, sb * ST : (sb + 1) * ST, :], vps)
                return qt

            def stage3(sb, qt, tail=None):
                """Causal attention for q-block sb, all 4 heads.
                Emission is software-pipelined: DEPTH score matmuls run ahead
                of the exp->l/PV chain so the in-order PE stream never stalls
                on ACT latency. Off-diagonal P tiles are pair-summed on the
                DVE so the l (ones) matmul streams half the columns."""
                ot = otp.tile([128, NREP, 512], bf16, tag="ot")
                nkt = (sb + 1) * ST
                DEPTH = 4

                def kt_geo(kt):
                    """Valid q range for k-tile kt in this q-block: diagonal
                    tiles only cover q >= 128*r (causal width trim)."""
                    r = kt - sb * ST
                    qo = 128 * r if r > 0 else 0
                    return r, qo

                def emit_st(h, kt):
                    r, qo = kt_geo(kt)
                    pst = psS.tile([128, 512], f32, tag="st")
                    nc.tensor.matmul(
                        pst[:, qo:],
                        KT[:, kt // ST, (kt % ST) * 128 : (kt % ST + 1) * 128],
                        qt[:, h, qo:],
                        start=True, stop=(r < 0),
                    )
                    if r >= 0:
                        # causal mask: PE-accumulate mneg^T (-1e5 above the
                        # diagonal) onto the 128-col strip; exp then
                        # underflows the masked entries to 0
                        nc.tensor.matmul(
                            pst[:, qo : qo + 128], mnegT_s, ident_s,
                            start=False, stop=True, skip_group_check=True,
                        )
                    return pst

                # score prefetch runs DEPTH items ahead ACROSS head
                # boundaries, so head transitions have no pipeline refill
                items = [(h, kt) for h in range(NREP) for kt in range(nkt)]
                sts = {}
                ahead = 0

                def prefetch(upto):
                    nonlocal ahead
                    while ahead < len(items) and ahead < upto:
                        sts[items[ahead]] = emit_st(*items[ahead])
                        ahead += 1

                po = pl = None
                lfirst = True
                quad = []
                for idx, (h, kt) in enumerate(items):
                    if kt == 0:
                        po = psA.tile([128, 512], f32, tag="o", bufs=1)
                        pl = psB.tile([128, 512], f32, tag="l")
                        lfirst = True
                        quad = []
                    prefetch(idx + DEPTH)
                    r, qo = kt_geo(kt)
                    pst = sts.pop((h, kt))
                    pt = ptp.tile([128, 512], bf16, tag="pt")
                    nc.scalar.activation(pt[:, qo:], pst[:, qo:], EXP, scale=SCALE)
                    if r < 0:
                        # off-diagonal: quad-sum P tiles on the DVE so the
                        # ones (l) matmul streams 1/4 of the columns
                        quad.append(pt)
                        if len(quad) == 4:
                            p01 = ppp.tile([128, 512], bf16, tag="pp")
                            nc.vector.tensor_add(p01, quad[0], quad[1])
                            p23 = ppp.tile([128, 512], bf16, tag="pp")
                            nc.vector.tensor_add(p23, quad[2], quad[3])
                            p03 = ppp.tile([128, 512], bf16, tag="pp")
                            nc.vector.tensor_add(p03, p01, p23)
                            quad = []
                            nc.tensor.matmul(
                                pl, ones_s, p03, start=lfirst, stop=False,
                            )
                            lfirst = False
                    else:
                        nc.tensor.matmul(
                            pl[:, qo:], ones_s, pt[:, qo:],
                            start=lfirst, stop=(kt == nkt - 1),
                        )
                        lfirst = False
                    nc.tensor.matmul(
                        po[:, qo:], V[:, kt, :], pt[:, qo:],
                        start=(kt == 0), stop=(kt == nkt - 1),
                    )
                    if kt == nkt - 1:
                        rb = rp.tile([128, 512], f32, tag="rb")
                        nc.vector.reciprocal_approx_fast(out=rb, in_=pl)
                        nc.vector.tensor_mul(ot[:, h, :], po, rb)
                        if tail is not None:
                            # emit deferred PE work (previous block's wo
                            # stage) under this ACT-bound stretch
                            tail(h)
                return ot

            def stage4_st(sb, ot, st):
                """One row-block of the output projection for q-block sb
                (bf16 partials out, casts on DVE, two out DMAs per row)."""
                ob = obp.tile([128, 4, 512], bf16, tag="ob")
                for db in range(4):
                    pw = psA.tile([128, 512], f32, tag="proj")
                    for h in range(NREP):
                        nc.tensor.matmul(
                            pw,
                            ot[:, h, st * 128 : (st + 1) * 128],
                            wo_s[:, h, db * 512 : (db + 1) * 512],
                            start=(h == 0), stop=(h == NREP - 1),
                        )
                    nc.vector.tensor_copy(ob[:, db, :], pw)
                row0 = (sb * ST + st) * 128
                nc.sync.dma_start(
                    out_d.ap()[row0 : row0 + 128, 0:1024], ob[:, 0:2, :]
                )
                nc.sync.dma_start(
                    out_d.ap()[row0 : row0 + 128, 1024:2048], ob[:, 2:4, :]
                )

            def stage4(sb, ot):
                for st in range(ST):
                    stage4_st(sb, ot, st)

            # ---- ordered DMA prologue. DMA queue completion is in-order and
            # Sync posting is ~0.7us/DMA, so deliveries are ordered to match
            # stage2(0)'s consumption: wq h0 + x^T block 0 first, then cs,
            # then wq h1..h3 (one per ~6us of h-loop), then wkv. ----
            wq_ap = wq_d.ap().rearrange("(t k) e -> k t e", k=128)

            def load_wq(h):
                nc.sync.dma_start(
                    wq_s[:, :, h * 128 : (h + 1) * 128],
                    wq_ap[:, :, h * 128 : (h + 1) * 128],
                )

            # delivery ordered by stage2(0)'s h-major consumption, with the
            # h0 pass's inputs (wq h0, x^T tiles) split into 256KB chunks so
            # the first matmul starts ~9.5us and the h0 pass tracks delivery;
            # then wq h1..h3 (one per ~6us of h-loop), wkv for the K/V passes
            nc.sync.dma_start(wq_s[:, 0:8, 0:128], wq_ap[:, 0:8, 0:128])
            xt0 = xtp.tile([128, DT, 512], bf16, tag="xt")
            for dg in range(3):
                nc.sync.dma_start(
                    xt0[:, 2 * dg : 2 * dg + 2, :],
                    xT_ap[:, 2 * dg : 2 * dg + 2, 0:512],
                )
            nc.sync.dma_start(wq_s[:, 8:16, 0:128], wq_ap[:, 8:16, 0:128])
            for dg in range(3, 8):
                nc.sync.dma_start(
                    xt0[:, 2 * dg : 2 * dg + 2, :],
                    xT_ap[:, 2 * dg : 2 * dg + 2, 0:512],
                )
            xts = [xt0]
            load_wq(1)
            nc.sync.dma_start(cs_s, cs_d.ap())
            load_wq(2)
            load_wq(3)
            nc.sync.dma_start(wkv_s, wkv_d.ap().rearrange("(t k) e -> k t e", k=128))
            nc.sync.dma_start(moi_s, moi_d.ap())
            xts.append(load_xt(1, 2))
            xts.append(load_xt(2, 2))
            xts.append(load_xt(3, 2))
            wo_ap = wo_d.ap().rearrange("(h k) n -> k h n", k=128)
            for i in range(2):
                nc.sync.dma_start(
                    wo_s[:, :, 1024 * i : 1024 * i + 1024],
                    wo_ap[:, :, 1024 * i : 1024 * i + 1024],
                )
            # Software-pipelined outer loop: projections for block sb+1 are
            # emitted BEFORE the wo-stage of block sb, so the in-order PE
            # stream has independent matmuls to run while block sb's
            # normalization tail completes. The wo-stage of block 2 is
            # deferred INTO block 3's attention (stage3(3) is ACT-bound:
            # ~41us of exp vs ~30us of PE per core), filling the PE there.
            qt = stage2(0, xts[0])
            ots = {}
            for sb in range(SB - 1):
                ots[sb] = stage3(sb, qt)
                if sb + 1 < SB:
                    qt = stage2(sb + 1, xts[sb + 1])
                if sb < SB - 2:
                    stage4(sb, ots[sb])
            ot3 = stage3(
                SB - 1, qt, tail=lambda h: stage4_st(SB - 2, ots[SB - 2], h)
            )
            stage4(SB - 1, ot3)
    nc.finalize()
    return nc


def _get_nc():
    if "nc" not in _CACHE:
        _CACHE["nc"] = _build()
    return _CACHE["nc"]


def _host_prep(x, wq, wk, wv, wo, freqs_cos, freqs_sin):
    """Build the 8 per-core input maps (bf16 casts + x transpose on host)."""
    import ml_dtypes

    bf = ml_dtypes.bfloat16
    perm = np.concatenate([np.arange(0, HD, 2), np.arange(1, HD, 2)])  # even|odd
    csT = np.concatenate(
        [np.ascontiguousarray(freqs_cos.T), np.ascontiguousarray(freqs_sin.T)], axis=0
    ).astype(bf)  # [128, S]
    # mneg^T[i,j] = -1e5 strictly above the diagonal (masked, q < k within
    # the diagonal strip after the PE transpose-accumulate), else 0
    ii = np.arange(128, dtype=np.int64)[:, None]
    jj = np.arange(128, dtype=np.int64)[None, :]
    mnegT = np.where(ii >= jj, 0.0, -1e5).astype(np.float32)
    moi = np.concatenate(
        [mnegT, np.ones((128, 128), np.float32), np.eye(128, dtype=np.float32)],
        axis=1,
    ).astype(bf)

    xT = [np.ascontiguousarray(x[b].astype(bf).T) for b in range(B)]
    in_maps = []
    for c in range(NC_CORES):
        b, g = divmod(c, NREP)
        wq_g = wq[:, g * EG : (g + 1) * EG].copy()
        for h in range(NREP):
            blk = wq_g[:, h * HD : (h + 1) * HD]
            wq_g[:, h * HD : (h + 1) * HD] = blk[:, perm]
        wk_g = wk[:, g * HD : (g + 1) * HD][:, perm]
        wv_g = wv[:, g * HD : (g + 1) * HD]
        wkv_g = np.concatenate([wk_g, wv_g], axis=1)
        wo_g = wo[g * EG : (g + 1) * EG, :]
        in_maps.append(
            {
                "xT": xT[b],
                "wq": np.ascontiguousarray(wq_g).astype(bf),
                "wkv": np.ascontiguousarray(wkv_g).astype(bf),
                "wo": np.ascontiguousarray(wo_g).astype(bf),
                "csT": csT,
                "moi": moi,
            }
        )
    return in_maps


def kernel(x, wq, wk, wv, wo, freqs_cos, freqs_sin):
    global LAST_RESULT
    from concourse.bass_utils import run_bass_kernel_spmd

    trace = bool(int(os.environ.get("BASS_KERNEL_TRACE", "0")))
    if trace:
        _install_trace_shim()

    x = np.asarray(x, dtype=np.float32)
    wq = np.asarray(wq, dtype=np.float32)
    wk = np.asarray(wk, dtype=np.float32)
    wv = np.asarray(wv, dtype=np.float32)
    wo = np.asarray(wo, dtype=np.float32)
    freqs_cos = np.asarray(freqs_cos, dtype=np.float32)
    freqs_sin = np.asarray(freqs_sin, dtype=np.float32)

    nc = _get_nc()
    in_maps = _host_prep(x, wq, wk, wv, wo, freqs_cos, freqs_sin)
    res = run_bass_kernel_spmd(nc, in_maps, list(range(NC_CORES)), trace=trace)
    LAST_RESULT = res

    out = np.empty((B, S, D), dtype=np.float32)
    for b in range(B):
        acc = res.results[b * NREP]["out"].astype(np.float32, copy=True)
        for g in range(1, NREP):
            acc += res.results[b * NREP + g]["out"].astype(np.float32)
        out[b] = acc
    return out


# revision 35
# speedup vs baseline: 1.0254x; 1.0037x over previous
"""GQA attention (B=2, S=2048, D=2048, H=16, KV=4, HD=128) with RoPE + causal
softmax + output projection, on 8 TRN2 NeuronCores.

Sharding: B x KV = 2 x 4 = 8 perfectly balanced shards. Core c handles batch
c//4 and kv-group c%4 (4 q heads + 1 kv head). wq/wk/wv are column-sharded,
wo row-sharded; the 4 partial wo outputs per batch are summed on the host
(the unshard step for a row-sharded matmul).

Design (v3, from the v1 306us baseline; measured 244.7us traced at the
fast DVFS state, rel err 6.2e-3):
  - x is transposed on the HOST, so x^T tiles arrive via plain contiguous
    DMA (2KB packets) instead of the transpose crossbar (256B packets).
    v1 lost ~30us of prologue and a ~30us mid-kernel stall to transpose
    DMA contention.
  - prologue DMAs are ordered by stage2(0)'s h-major consumption (wq h0,
    x^T block 0, cs, wq h1..h3, wkv); DMA queue completion is in-order
    and Sync posts ~0.7us/DMA, so order is everything. First matmul ~11us
    (7us of that is fixed framework init).
  - V tiles ([s,hd] layout) produced by PE identity-transposes of the
    bf16 pv projection (start/stop groups into one PSUM bank), then one
    ACT copy per block to SBUF. No SBUF->SBUF transpose DMA.
  - causal mask via a PE-accumulated mneg^T (-1e5 above diagonal) matmul
    onto the diagonal score strip, keeping DVE/GpSimd off the
    scores->exp->l/PV critical path; exp then underflows masked to 0.
  - full-bf16 matmul pipeline (f32 PSUM accumulation), exp on ACT with no
    max subtraction (scores are O(1) by construction).
  - l via all-ones [128,128] matmul (broadcasts sum_k P into all
    partitions, PSUM-accumulated). Off-diagonal P tiles are quad-summed
    on the DVE first, so the ones-matmul streams 1/4 of the columns
    (PE cols for l: 69.6k -> 32.8k per core).
  - causal width trim: diagonal score tiles only compute q >= 128*r.
  - score matmuls prefetch DEPTH=4 items ahead ACROSS head boundaries
    (no per-head pipeline refill); ACT exp is the stage3 near-bottleneck
    so stage4's PSUM->SBUF casts all go to DVE, and block 2's wo-stage
    (pure PE) is deferred into block 3's ACT-bound attention.
  - out partials stored/DMA'd as bf16 (halves output traffic); host
    accumulates the 4 row-shard partials in f32.
  - PSUM banks: proj(2, shared w/ V-transpose + stage4) + o(1) + st(4) +
    l(1) = 8.
  - NOTE: run-to-run HW exec varies ~±10% with the chip's DVFS state
    (matmul slice 379ns vs 454ns mode); compare min-of-N or
    clock-normalized.
"""
import os
import sys

import numpy as np

if "/opt/trn_rl_repo" not in sys.path:
    sys.path.insert(0, "/opt/trn_rl_repo")

B, S, D = 2, 2048, 2048
H, KV, HD = 16, 4, 128
NREP = H // KV            # 4 q heads per core
EG = NREP * HD            # 512: per-core q width
NC_CORES = 8
SB = 4                    # seq blocks of 512
ST = 4                    # 128-row seq tiles per block
DT = D // 128             # 16 contraction tiles
SCALE = float(1.0 / np.sqrt(HD))

_CACHE = {}
LAST_RESULT = None        # BassKernelResults of the most recent run (for test.py)


def _install_trace_shim():
    """antenv.axon_hooks is missing in this image; run_bass_kernel_spmd's
    trace path needs it. Also neuter the S3 artifact upload."""
    import types

    try:
        import antenv.axon_hooks  # noqa: F401
    except ImportError:
        try:
            import antenv
            from trn_agent_boot.trn_boot import _ntff_profile_via_ctypes

            mod = types.ModuleType("antenv.axon_hooks")
            _hook = [None]
            mod.set_axon_ntff_profile_hook = lambda h: _hook.__setitem__(0, h)
            mod.get_axon_ntff_profile_hook = lambda: _hook[0]
            sys.modules["antenv.axon_hooks"] = mod
            antenv.axon_hooks = mod
            mod.set_axon_ntff_profile_hook(
                _ntff_profile_via_ctypes("/opt/axon/libaxon_pjrt.so")
            )
        except Exception:
            return
    import concourse.bass_utils as bu

    bu.upload_artifacts = lambda tmpdir: f"local:{tmpdir}"


def _build():
    import concourse.mybir as mybir
    import concourse.tile as tile
    from concourse import bacc

    f32 = mybir.dt.float32
    bf16 = mybir.dt.bfloat16
    EXP = mybir.ActivationFunctionType.Exp

    nc = bacc.Bacc(None, target_bir_lowering=False)
    xT_d = nc.declare_dram_parameter("xT", [D, S], bf16, isOutput=False)
    wq_d = nc.declare_dram_parameter("wq", [D, EG], bf16, isOutput=False)
    wkv_d = nc.declare_dram_parameter("wkv", [D, 2 * HD], bf16, isOutput=False)
    wo_d = nc.declare_dram_parameter("wo", [EG, D], bf16, isOutput=False)
    cs_d = nc.declare_dram_parameter("csT", [128, S], bf16, isOutput=False)
    moi_d = nc.declare_dram_parameter("moi", [128, 384], bf16, isOutput=False)
    out_d = nc.declare_dram_parameter("out", [S, D], bf16, isOutput=True)

    with tile.TileContext(nc) as tc:
        with (
            tc.tile_pool(name="fixed", bufs=1) as fixed,
            tc.tile_pool(name="xt", bufs=4) as xtp,
            tc.tile_pool(name="qt", bufs=2) as qtp,
            tc.tile_pool(name="ot", bufs=2) as otp,
            tc.tile_pool(name="pt", bufs=6) as ptp,
            tc.tile_pool(name="rope", bufs=3) as ropep,
            tc.tile_pool(name="pp", bufs=3) as ppp,
            tc.tile_pool(name="vt", bufs=2) as vtp,
            tc.tile_pool(name="r", bufs=2) as rp,
            tc.tile_pool(name="ob", bufs=3) as obp,
            # PSUM banks: proj(2) + o(1) + st(4) + l(1) = 8
            tc.tile_pool(name="psA", bufs=2, space="PSUM") as psA,
            tc.tile_pool(name="psS", bufs=4, space="PSUM") as psS,
            tc.tile_pool(name="psB", bufs=1, space="PSUM") as psB,
        ):
            # ---- persistent tiles (DMAs emitted in the ordered prologue) ----
            wq_s = fixed.tile([128, DT, EG], bf16)
            wkv_s = fixed.tile([128, DT, 2 * HD], bf16)
            wo_s = fixed.tile([128, NREP, D], bf16)
            # moi = mneg^T (strict upper-tri -1e5) | all-ones | identity.
            # mneg^T is PE-accumulated onto diagonal score strips (mask add on
            # the PE keeps the mask off the DVE/ACT critical path). The
            # l-matmul with ones broadcasts sum_k P into every output
            # partition at the same cost as an M=1 matmul (cost ~ N), making
            # 1/l directly consumable by the O^T normalize multiply.
            moi_s = fixed.tile([128, 384], bf16)
            mnegT_s = moi_s[:, 0:128]
            ones_s = moi_s[:, 128:256]
            ident_s = moi_s[:, 256:384]
            cs_s = fixed.tile([128, S], bf16)       # cos^T | sin^T, all blocks
            KT = fixed.tile([128, SB, 512], bf16)   # rotated K^T [hd, s]
            V = fixed.tile([128, DT, HD], bf16)     # V [s%128, s-tile, hd]

            def rope(dst, psrc, cs):
                """dst[128,512] bf16 = rotate(psrc[128,512] PSUM f32).
                Rows 0:64 = real half, 64:128 = imag half (pre-permuted
                weights); cs rows 0:64 = cos^T, 64:128 = sin^T. Multiplies
                on DVE (PSUM reads), add/sub on GpSimd (SBUF only)."""
                re, im = psrc[0:64, :], psrc[64:128, :]
                co, si = cs[0:64, :], cs[64:128, :]
                t1 = ropep.tile([64, 512], bf16, tag="t1")
                nc.vector.tensor_mul(t1, re, co)
                t2 = ropep.tile([64, 512], bf16, tag="t2")
                nc.vector.tensor_mul(t2, im, si)
                nc.gpsimd.tensor_sub(dst[0:64, :], t1, t2)
                t3 = ropep.tile([64, 512], bf16, tag="t1")
                nc.vector.tensor_mul(t3, re, si)
                t4 = ropep.tile([64, 512], bf16, tag="t2")
                nc.vector.tensor_mul(t4, im, co)
                nc.gpsimd.tensor_add(dst[64:128, :], t3, t4)

            xT_ap = xT_d.ap().rearrange("(t k) s -> k t s", k=128)

            def load_xt(sb, nchunk):
                """x^T for block sb: plain contiguous DMA (host transposed)."""
                xt = xtp.tile([128, DT, 512], bf16, tag="xt")
                step = DT // nchunk
                for dg in range(nchunk):
                    nc.sync.dma_start(
                        xt[:, dg * step : (dg + 1) * step, :],
                        xT_ap[
                            :, dg * step : (dg + 1) * step, sb * 512 : (sb + 1) * 512
                        ],
                    )
                return xt

            def stage2(sb, xt):
                """Q^T/K^T/V projections + RoPE for block sb."""
                cs = cs_s[:, sb * 512 : (sb + 1) * 512]
                qt = qtp.tile([128, NREP, 512], bf16, tag="qt")
                for h in range(NREP):
                    pq = psA.tile([128, 512], f32, tag="proj")
                    for dt in range(DT):
                        nc.tensor.matmul(
                            pq,
                            wq_s[:, dt, h * 128 : (h + 1) * 128],
                            xt[:, dt, :],
                            start=(dt == 0),
                            stop=(dt == DT - 1),
                        )
                    rope(qt[:, h, :], pq, cs)

                pk = psA.tile([128, 512], f32, tag="proj")
                for dt in range(DT):
                    nc.tensor.matmul(
                        pk, wkv_s[:, dt, 0:HD], xt[:, dt, :],
                        start=(dt == 0), stop=(dt == DT - 1),
                    )
                rope(KT[:, sb, :], pk, cs)

                pv = psA.tile([128, 512], f32, tag="proj")
                for dt in range(DT):
                    nc.tensor.matmul(
                        pv, wkv_s[:, dt, HD : 2 * HD], xt[:, dt, :],
                        start=(dt == 0), stop=(dt == DT - 1),
                    )
                vt_tmp = vtp.tile([128, 512], bf16, tag="vt")
                nc.vector.tensor_copy(vt_tmp, pv)
                # PE identity-transpose pv^T -> V[s,hd] tiles (one PSUM bank,
                # 4 independent start/stop groups into disjoint regions).
                vps = psA.tile([128, ST, HD], bf16, tag="proj")
                for c in range(ST):
                    nc.tensor.transpose(
                        vps[:, c, :], vt_tmp[:, c * 128 : (c + 1) * 128], ident_s
                    )
                nc.scalar.copy(V[:, sb * ST : (sb + 1) * ST, :], vps)
                return qt

            def stage3(sb, qt, tail=None):
                """Causal attention for q-block sb, all 4 heads.
                Emission is software-pipelined: DEPTH score matmuls run ahead
                of the exp->l/PV chain so the in-order PE stream never stalls
                on ACT latency. Off-diagonal P tiles are pair-summed on the
                DVE so the l (ones) matmul streams half the columns."""
                ot = otp.tile([128, NREP, 512], bf16, tag="ot")
                nkt = (sb + 1) * ST
                DEPTH = 4

                def kt_geo(kt):
                    """Valid q range for k-tile kt in this q-block: diagonal
                    tiles only cover q >= 128*r (causal width trim)."""
                    r = kt - sb * ST
                    qo = 128 * r if r > 0 else 0
                    return r, qo

                def emit_st(h, kt):
                    r, qo = kt_geo(kt)
                    pst = psS.tile([128, 512], f32, tag="st")
                    nc.tensor.matmul(
                        pst[:, qo:],
                        KT[:, kt // ST, (kt % ST) * 128 : (kt % ST + 1) * 128],
                        qt[:, h, qo:],
                        start=True, stop=(r < 0),
                    )
                    if r >= 0:
                        # causal mask: PE-accumulate mneg^T (-1e5 above the
                        # diagonal) onto the 128-col strip; exp then
                        # underflows the masked entries to 0
                        nc.tensor.matmul(
                            pst[:, qo : qo + 128], mnegT_s, ident_s,
                            start=False, stop=True, skip_group_check=True,
                        )
                    return pst

                # score prefetch runs DEPTH items ahead ACROSS head
                # boundaries, so head transitions have no pipeline refill
                items = [(h, kt) for h in range(NREP) for kt in range(nkt)]
                sts = {}
                ahead = 0

                def prefetch(upto):
                    nonlocal ahead
                    while ahead < len(items) and ahead < upto:
                        sts[items[ahead]] = emit_st(*items[ahead])
                        ahead += 1

                po = pl = None
                lfirst = True
                quad = []
                for idx, (h, kt) in enumerate(items):
                    if kt == 0:
                        po = psA.tile([128, 512], f32, tag="o", bufs=1)
                        pl = psB.tile([128, 512], f32, tag="l")
                        lfirst = True
                        quad = []
                    prefetch(idx + DEPTH)
                    r, qo = kt_geo(kt)
                    pst = sts.pop((h, kt))
                    pt = ptp.tile([128, 512], bf16, tag="pt")
                    nc.scalar.activation(pt[:, qo:], pst[:, qo:], EXP, scale=SCALE)
                    if r < 0:
                        # off-diagonal: quad-sum P tiles on the DVE so the
                        # ones (l) matmul streams 1/4 of the columns
                        quad.append(pt)
                        if len(quad) == 4:
                            p01 = ppp.tile([128, 512], bf16, tag="pp")
                            nc.vector.tensor_add(p01, quad[0], quad[1])
                            p23 = ppp.tile([128, 512], bf16, tag="pp")
                            nc.vector.tensor_add(p23, quad[2], quad[3])
                            p03 = ppp.tile([128, 512], bf16, tag="pp")
                            nc.vector.tensor_add(p03, p01, p23)
                            quad = []
                            nc.tensor.matmul(
                                pl, ones_s, p03, start=lfirst, stop=False,
                            )
                            lfirst = False
                    else:
                        nc.tensor.matmul(
                            pl[:, qo:], ones_s, pt[:, qo:],
                            start=lfirst, stop=(kt == nkt - 1),
                        )
                        lfirst = False
                    nc.tensor.matmul(
                        po[:, qo:], V[:, kt, :], pt[:, qo:],
                        start=(kt == 0), stop=(kt == nkt - 1),
                    )
                    if kt == nkt - 1:
                        rb = rp.tile([128, 512], f32, tag="rb")
                        nc.vector.reciprocal_approx_fast(out=rb, in_=pl)
                        nc.vector.tensor_mul(ot[:, h, :], po, rb)
                        if tail is not None:
                            # emit deferred PE work (previous block's wo
                            # stage) under this ACT-bound stretch
                            tail(h)
                return ot

            def stage4_st(sb, ot, st):
                """One row-block of the output projection for q-block sb
                (bf16 partials out, casts on DVE, two out DMAs per row)."""
                ob = obp.tile([128, 4, 512], bf16, tag="ob")
                for db in range(4):
                    pw = psA.tile([128, 512], f32, tag="proj")
                    for h in range(NREP):
                        nc.tensor.matmul(
                            pw,
                            ot[:, h, st * 128 : (st + 1) * 128],
                            wo_s[:, h, db * 512 : (db + 1) * 512],
                            start=(h == 0), stop=(h == NREP - 1),
                        )
                    nc.vector.tensor_copy(ob[:, db, :], pw)
                row0 = (sb * ST + st) * 128
                nc.sync.dma_start(
                    out_d.ap()[row0 : row0 + 128, 0:1024], ob[:, 0:2, :]
                )
                nc.sync.dma_start(
                    out_d.ap()[row0 : row0 + 128, 1024:2048], ob[:, 2:4, :]
                )

            def stage4(sb, ot):
                for st in range(ST):
                    stage4_st(sb, ot, st)

            # ---- ordered DMA prologue. DMA queue completion is in-order and
            # Sync posting is ~0.7us/DMA, so deliveries are ordered to match
            # stage2(0)'s consumption: wq h0 + x^T block 0 first, then cs,
            # then wq h1..h3 (one per ~6us of h-loop), then wkv. ----
            wq_ap = wq_d.ap().rearrange("(t k) e -> k t e", k=128)

            def load_wq(h):
                nc.sync.dma_start(
                    wq_s[:, :, h * 128 : (h + 1) * 128],
                    wq_ap[:, :, h * 128 : (h + 1) * 128],
                )

            # delivery ordered by stage2(0)'s h-major consumption, with the
            # h0 pass's inputs (wq h0, x^T tiles) split into 256KB chunks so
            # the first matmul starts ~9.5us and the h0 pass tracks delivery;
            # then wq h1..h3 (one per ~6us of h-loop), wkv for the K/V passes
            nc.sync.dma_start(wq_s[:, 0:8, 0:128], wq_ap[:, 0:8, 0:128])
            xt0 = xtp.tile([128, DT, 512], bf16, tag="xt")
            for dg in range(3):
                nc.sync.dma_start(
                    xt0[:, 2 * dg : 2 * dg + 2, :],
                    xT_ap[:, 2 * dg : 2 * dg + 2, 0:512],
                )
            nc.sync.dma_start(wq_s[:, 8:16, 0:128], wq_ap[:, 8:16, 0:128])
            for dg in range(3, 8):
                nc.sync.dma_start(
                    xt0[:, 2 * dg : 2 * dg + 2, :],
                    xT_ap[:, 2 * dg : 2 * dg + 2, 0:512],
                )
            xts = [xt0]
            load_wq(1)
            nc.sync.dma_start(cs_s, cs_d.ap())
            load_wq(2)
            load_wq(3)
            nc.sync.dma_start(wkv_s, wkv_d.ap().rearrange("(t k) e -> k t e", k=128))
            nc.sync.dma_start(moi_s, moi_d.ap())
            xts.append(load_xt(1, 2))
            xts.append(load_xt(2, 2))
            xts.append(load_xt(3, 2))
            wo_ap = wo_d.ap().rearrange("(h k) n -> k h n", k=128)
            for i in range(2):
                nc.sync.dma_start(
                    wo_s[:, :, 1024 * i : 1024 * i + 1024],
                    wo_ap[:, :, 1024 * i : 1024 * i + 1024],
                )
            # Software-pipelined outer loop: projections for block sb+1 are
            # emitted BEFORE the wo-stage of block sb, so the in-order PE
            # stream has independent matmuls to run while block sb's
            # normalization tail completes. The wo-stage of block 2 is
            # deferred INTO block 3's attention (stage3(3) is ACT-bound:
            # ~41us of exp vs ~30us of PE per core), filling the PE there.
            qt = stage2(0, xts[0])
            ots = {}
            for sb in range(SB - 1):
                ots[sb] = stage3(sb, qt)
                if sb + 1 < SB:
                    qt = stage2(sb + 1, xts[sb + 1])
                if sb < SB - 2:
                    stage4(sb, ots[sb])
            ot3 = stage3(
                SB - 1, qt, tail=lambda h: stage4_st(SB - 2, ots[SB - 2], h)
            )
            stage4(SB - 1, ot3)
    nc.finalize()
    return nc


def _get_nc():
    if "nc" not in _CACHE:
        _CACHE["nc"] = _build()
    return _CACHE["nc"]


def _host_prep(x, wq, wk, wv, wo, freqs_cos, freqs_sin):
    """Build the 8 per-core input maps (bf16 casts + x transpose on host)."""
    import ml_dtypes

    bf = ml_dtypes.bfloat16
    perm = np.concatenate([np.arange(0, HD, 2), np.arange(1, HD, 2)])  # even|odd
    csT = np.concatenate(
        [np.ascontiguousarray(freqs_cos.T), np.ascontiguousarray(freqs_sin.T)], axis=0
    ).astype(bf)  # [128, S]
    # mneg^T[i,j] = -1e5 strictly above the diagonal (masked, q < k within
    # the diagonal strip after the PE transpose-accumulate), else 0
    ii = np.arange(128, dtype=np.int64)[:, None]
    jj = np.arange(128, dtype=np.int64)[None, :]
    mnegT = np.where(ii >= jj, 0.0, -1e5).astype(np.float32)
    moi = np.concatenate(
        [mnegT, np.ones((128, 128), np.float32), np.eye(128, dtype=np.float32)],
        axis=1,
    ).astype(bf)

    xT = [np.ascontiguousarray(x[b].astype(bf).T) for b in range(B)]
    in_maps = []
    for c in range(NC_CORES):
        b, g = divmod(c, NREP)
        wq_g = wq[:, g * EG : (g + 1) * EG].copy()
        for h in range(NREP):
            blk = wq_g[:, h * HD : (h + 1) * HD]
            wq_g[:, h * HD : (h + 1) * HD] = blk[:, perm]
        wk_g = wk[:, g * HD : (g + 1) * HD][:, perm]
        wv_g = wv[:, g * HD : (g + 1) * HD]
        wkv_g = np.concatenate([wk_g, wv_g], axis=1)
        wo_g = wo[g * EG : (g + 1) * EG, :]
        in_maps.append(
            {
                "xT": xT[b],
                "wq": np.ascontiguousarray(wq_g).astype(bf),
                "wkv": np.ascontiguousarray(wkv_g).astype(bf),
                "wo": np.ascontiguousarray(wo_g).astype(bf),
                "csT": csT,
                "moi": moi,
            }
        )
    return in_maps


def kernel(x, wq, wk, wv, wo, freqs_cos, freqs_sin):
    global LAST_RESULT
    from concourse.bass_utils import run_bass_kernel_spmd

    trace = bool(int(os.environ.get("BASS_KERNEL_TRACE", "0")))
    if trace:
        _install_trace_shim()

    x = np.asarray(x, dtype=np.float32)
    wq = np.asarray(wq, dtype=np.float32)
    wk = np.asarray(wk, dtype=np.float32)
    wv = np.asarray(wv, dtype=np.float32)
    wo = np.asarray(wo, dtype=np.float32)
    freqs_cos = np.asarray(freqs_cos, dtype=np.float32)
    freqs_sin = np.asarray(freqs_sin, dtype=np.float32)

    nc = _get_nc()
    in_maps = _host_prep(x, wq, wk, wv, wo, freqs_cos, freqs_sin)
    res = run_bass_kernel_spmd(nc, in_maps, list(range(NC_CORES)), trace=trace)
    LAST_RESULT = res

    out = np.empty((B, S, D), dtype=np.float32)
    for b in range(B):
        acc = res.results[b * NREP]["out"].astype(np.float32, copy=True)
        for g in range(1, NREP):
            acc += res.results[b * NREP + g]["out"].astype(np.float32)
        out[b] = acc
    return out
